# revision 38
# baseline (speedup 1.0000x reference)
"""Trainium2 Bass kernel for 5-sweep Jacobi iteration (4th-order 2D Poisson).

Problem: B=16 samples of [1024,1024] f32; per-sample cross stencil from dx;
5 Jacobi sweeps; 2-wide boundary frame kept fixed at the initial guess.

Sharding: data-parallel over batch, 2 samples per core, 8 cores.

Layout: bf16 state, 9 row-blocks of 128 rows overlapping by 4 rows
(block b holds rows 124b..124b+128; block 8 holds rows 896..1024). Each
block computes out rows [2,126) locally (block 8: [98,126)) so the
H-direction taps never cross a block boundary -> no halo matmuls. The
4-row overlaps are kept coherent with small SBUF->SBUF DMAs per sweep.

Per [128,512] output unit and sweep:
  PE   : psum = Bc@x (H taps) + (-f1)(x<<1 + x>>1) + I@R     (4 matmuls)
  DVE/GPS: A2 = x<<2 + x>>2                                  (tensor_add)
  DVE  : nxt = (A2 * -f2) + psum                             (fused STT)
Boundary cols are never written (col-trimmed evac); boundary rows are
restored by DMA; host splices the exact fp32 boundary frame at the end.
"""

import sys

sys.path.insert(0, "/opt/trn_rl_repo")

import numpy as np
import ml_dtypes

BF = ml_dtypes.bfloat16

N_CORES = 8
B, H, W = 16, 1024, 1024
SPC = B // N_CORES  # samples per core
P = 128
OPB = 124            # out rows per block
NBLK = 9             # row blocks (8 full-stride + 1 tail)
BW = W + 4           # block width incl 2 guard cols each side
FREE = NBLK * BW     # 9252
RFREE = NBLK * W     # 9216
N_ITER = 5

_CACHE = {}


def _row_start(b):
    return 124 * b if b < NBLK - 1 else H - P  # block 8: rows 896..1024


def _host_coeffs(dx):
    """Per-sample stencil scalars in float64. dx: [B, 2]."""
    a = (1.0 / dx.astype(np.float64)) ** 2
    a0, a1 = a[:, 0], a[:, 1]
    dinv = 1.0 / (-2.5 * (a0 + a1))
    e1 = dinv * a0 * (4.0 / 3.0)
    e2 = dinv * a0 * (-1.0 / 12.0)
    f1 = dinv * a1 * (4.0 / 3.0)
    f2 = dinv * a1 * (-1.0 / 12.0)
    return dinv, e1, e2, f1, f2


def _host_mats(dx):
    """[B, 128, 512] lhsT mats: [Bc(-e taps) | -f1*I | I | -f2*I], -f2 scalars."""
    dinv, e1, e2, f1, f2 = _host_coeffs(dx)
    nb = dx.shape[0]
    mats = np.zeros((nb, P, 4 * P), np.float64)
    idx = np.arange(P)
    for s in range(nb):
        bc = mats[s, :, 0:P]
        for off, v in ((1, -e1[s]), (-1, -e1[s]), (2, -e2[s]), (-2, -e2[s])):
            kk = idx[(idx + off >= 0) & (idx + off < P)]
            bc[kk, kk + off] = v
        mats[s, :, P:2 * P][idx, idx] = -f1[s]
        mats[s, :, 2 * P:3 * P][idx, idx] = 1.0
        mats[s, :, 3 * P:4 * P][idx, idx] = -f2[s]
    scal = np.broadcast_to((-f2)[:, None, None], (nb, P, 1))
    return mats.astype(BF), np.ascontiguousarray(scal, dtype=np.float32), dinv


def _build_nc():
    import concourse.bacc as bacc
    import concourse.tile as tile
    from concourse import mybir

    f32 = mybir.dt.float32
    bf16 = mybir.dt.bfloat16
    nc = bacc.Bacc(
        "TRN2",
        target_bir_lowering=False,
        debug=False,
        enable_asserts=False,
        num_devices=N_CORES,
    )
    g_d = nc.dram_tensor("g", [SPC, P, FREE], bf16, kind="ExternalInput").ap()
    r_d = nc.dram_tensor("r", [SPC, P, RFREE], bf16, kind="ExternalInput").ap()
    m_d = nc.dram_tensor("m", [SPC, P, 4 * P], bf16, kind="ExternalInput").ap()
    c_d = nc.dram_tensor("c", [SPC, P, 1], f32, kind="ExternalInput").ap()
    o_d = nc.dram_tensor("o", [SPC, P, FREE], bf16, kind="ExternalOutput").ap()

    with tile.TileContext(nc) as tc:
        with (
            tc.tile_pool(name="state", bufs=1) as state,
            tc.tile_pool(name="tmp", bufs=11) as tmp,
            tc.tile_pool(name="psum", bufs=4, space="PSUM") as pp,
        ):
            gb = [
                [state.tile([P, FREE], bf16, name=f"g{s}_{i}", tag=f"g{s}_{i}")
                 for i in range(2)]
                for s in range(SPC)
            ]
            rb = [state.tile([P, RFREE], bf16, name=f"r{s}", tag=f"r{s}")
                  for s in range(SPC)]
            mt = [state.tile([P, 4 * P], bf16, name=f"m{s}", tag=f"m{s}")
                  for s in range(SPC)]
            cf = [state.tile([P, 1], f32, name=f"c{s}", tag=f"c{s}")
                  for s in range(SPC)]

            from concourse.ap import AP

            # the first compute block gates the whole pipeline: load it first,
            # split across both trigger queues
            nc.sync.dma_start(gb[0][0][:, 0:514], g_d[0][:, 0:514])
            nc.scalar.dma_start(gb[0][0][:, 514:BW], g_d[0][:, 514:BW])
            for s in range(SPC):
                nc.sync.dma_start(mt[s][:], m_d[s])
                nc.sync.dma_start(cf[s][:], c_d[s])
            nc.scalar.dma_start(rb[0][:, 0:W], r_d[0][:, 0:W])
            qs = [nc.sync, nc.scalar]
            qi = 0
            for s in range(SPC):
                for b in range(NBLK):
                    if s == 0 and b == 0:
                        continue
                    qs[qi % 2].dma_start(gb[s][0][:, BW * b: BW * (b + 1)],
                                         g_d[s][:, BW * b: BW * (b + 1)])
                    qs[(qi + 1) % 2].dma_start(rb[s][:, W * b: W * (b + 1)],
                                               r_d[s][:, W * b: W * (b + 1)])
                    qi += 1
            for s in range(SPC):
                # buffer 1 never gets a full load: evac rewrites everything
                # except the fixed boundary-col strips (offsets 2,3 / 1024,
                # 1025 per block) and block-8 rows 126,127. Source straight
                # from HBM so these never wait on the SBUF loads.
                gd = g_d[s]
                g1 = gb[s][1][:]
                strips = [[FREE, 128], [BW, NBLK], [1, 2]]
                for so in (2, 1024):
                    nc.sync.dma_start(
                        AP(tensor=g1.tensor, offset=g1.offset + so, ap=strips),
                        AP(tensor=gd.tensor, offset=gd.offset + so, ap=strips))
                # block 8 fully: partitions 0..95 are never evac'd and NaN
                # garbage there would poison the contraction
                nc.scalar.dma_start(
                    gb[s][1][:, BW * (NBLK - 1): BW * NBLK],
                    g_d[s][:, BW * (NBLK - 1): BW * NBLK])

            GPS_SET = (1, 4, 7)      # blocks whose A2 runs on gpsimd
            bidx = 0
            for it in range(N_ITER):
                last_it = it == N_ITER - 1
                for s in range(SPC):
                    cur = gb[s][it % 2]
                    nxt = gb[s][(it + 1) % 2]
                    border = range(NBLK)
                    if last_it and s == SPC - 1:
                        border = range(NBLK - 1, -1, -1)  # drain stores early
                    # front-load all A2 adds so evacs never wait on them
                    a2s = {}
                    for b in border:
                        a2 = tmp.tile([P, 1024], bf16, name="a2", tag="a2")
                        a2s[b] = a2
                        bof = BW * b
                        eng = (nc.gpsimd if (bidx + b) % 9 in GPS_SET
                               else nc.vector)
                        eng.tensor_add(a2[:, 0:1020],
                                       cur[:, bof + 2: bof + 1022],
                                       cur[:, bof + 6: bof + 1026])
                    bidx += 1
                    for b in border:
                        a2 = a2s[b]
                        ps = pp.tile([P, 1024], f32, name="ps", tag="ps")
                        bof = BW * b
                        p0, psz = (96, 30) if b == NBLK - 1 else (0, 126)
                        # PE: H banded + f1 shifts + R inject, per 512-half
                        for h2 in range(2):
                            base = bof + 2 + 512 * h2
                            po = 512 * h2
                            nc.tensor.matmul(ps[:, po:po + 512], mt[s][:, 0:P],
                                             cur[:, base: base + 512],
                                             start=True, stop=False,
                                             skip_group_check=True)
                            for d in (-1, 1):
                                nc.tensor.matmul(ps[:, po:po + 512],
                                                 mt[s][:, P:2 * P],
                                                 cur[:, base + d: base + d + 512],
                                                 start=False, stop=False,
                                                 skip_group_check=True)
                            nc.tensor.matmul(ps[:, po:po + 512],
                                             mt[s][:, 2 * P:3 * P],
                                             rb[s][:, W * b + po:
                                                   W * b + po + 512],
                                             start=False, stop=True,
                                             skip_group_check=True)
                        # evac: nxt = (A2 * -f2) + psum
                        nc.vector.scalar_tensor_tensor(
                            nxt[p0:p0 + psz, bof + 4: bof + 1024],
                            a2[p0:p0 + psz, 0:1020],
                            cf[s][p0:p0 + psz, 0:1],
                            ps[p0:p0 + psz, 2:1022],
                            op0=mybir.AluOpType.mult,
                            op1=mybir.AluOpType.add,
                        )
                    if last_it:
                        continue  # output uses interior rows only
                    # overlap-row maintenance for next sweep (batched)
                    v = nxt[:].rearrange("p (b w) -> p b w", b=NBLK)
                    nc.sync.dma_start(v[0:2, 1:8, :], v[124:126, 0:7, :])
                    nc.scalar.dma_start(v[126:128, 0:7, :], v[2:4, 1:8, :])
                    nc.sync.dma_start(
                        nxt[96:98, BW * 8: BW * 9], nxt[124:126, BW * 7: BW * 8])
                    nc.scalar.dma_start(
                        nxt[126:128, BW * 7: BW * 8], nxt[98:100, BW * 8: BW * 9])
                    # restore fixed global rows 0,1 (block 0)
                    nc.scalar.dma_start(nxt[0:2, 0:BW], cur[0:2, 0:BW])

            for s in range(SPC):
                final = gb[s][N_ITER % 2]
                if s == SPC - 1:
                    # fine-grained reversed drain: the last store is one block
                    groups = ((6, NBLK), (3, 6), (2, 3), (1, 2), (0, 1))
                else:
                    groups = ((0, 3), (3, 6), (6, NBLK))
                for gi, (lo, hi) in enumerate(groups):
                    qs[gi % 2].dma_start(o_d[s][:, BW * lo: BW * hi],
                                        final[:, BW * lo: BW * hi])

    nc.compile()
    return nc


def _get_nc():
    if "nc" not in _CACHE:
        _CACHE["nc"] = _build_nc()
    return _CACHE["nc"]


def _to_blocks(x, width, guard):
    """[B, H, W(+0)] f32 -> [B, P, NBLK*(W+2*guard)] bf16 with row overlap."""
    nb = x.shape[0]
    out = np.zeros((nb, P, NBLK * (width + 2 * guard)), BF)
    for b in range(NBLK):
        rs = _row_start(b)
        sl = out[:, :, b * (width + 2 * guard) + guard:
                 (b + 1) * (width + 2 * guard) - guard]
        sl[:] = x[:, rs:rs + P, :].astype(BF)
    return out


def kernel(current_guess, rhses, dx):
    from concourse.bass_utils import run_bass_kernel_spmd

    g32 = np.ascontiguousarray(current_guess[:, 0], dtype=np.float32)
    r32 = np.ascontiguousarray(rhses[:, 0], dtype=np.float32)
    mats, scal, dinv = _host_mats(dx)
    g = _to_blocks(g32, W, 2)
    r = _to_blocks(r32 * dinv[:, None, None].astype(np.float32), W, 0)

    nc = _get_nc()
    in_maps = []
    for c in range(N_CORES):
        sl = slice(c * SPC, (c + 1) * SPC)
        in_maps.append({
            "g": np.ascontiguousarray(g[sl]).view(np.uint16),
            "r": np.ascontiguousarray(r[sl]).view(np.uint16),
            "m": np.ascontiguousarray(mats[sl]).view(np.uint16),
            "c": np.ascontiguousarray(scal[sl]),
        })
    res = run_bass_kernel_spmd(nc, in_maps, core_ids=list(range(N_CORES)))
    _CACHE["last_results"] = res
    ob = np.concatenate([res.results[c]["o"] for c in range(N_CORES)], axis=0)
    blk = ob.view(BF).astype(np.float32).reshape(B, P, NBLK, BW).transpose(0, 2, 1, 3)

    out = np.empty((B, H, W), np.float32)
    for b in range(NBLK - 1):
        out[:, 124 * b + 2: 124 * b + 126, :] = blk[:, b, 2:126, 2:2 + W]
    out[:, 994:1022, :] = blk[:, NBLK - 1, 98:126, 2:2 + W]
    # exact fp32 boundary frame from the input
    out[:, 0:2, :] = g32[:, 0:2, :]
    out[:, 1022:1024, :] = g32[:, 1022:1024, :]
    out[:, :, 0:2] = g32[:, :, 0:2]
    out[:, :, 1022:1024] = g32[:, :, 1022:1024]
    return out[:, None].astype(np.float32)
